# revision 1
# baseline (speedup 1.0000x reference)
"""CodebookLoRASTELinear forward on 8 Trainium2 NeuronCores.

out = x @ (W_q + D)^T
  D   = (lora_B @ lora_A) * (alpha/rank)
  cb  = codebook / max|codebook|
  S   = exp(scale_log)                     (per [o, i//128] group)
  q   = cb[searchsorted(midpoints(cb), (W+D)/S)]
      == cb0 + sum_k d_k * ((W+D) > t_k*S)      (S > 0)
  W_q = q * S

Column-parallel sharding: W / scale / lora_B rows (out_features) are split
across the 8 cores; x and lora_A are replicated; per-core outputs are
concatenated on the host (no collectives).

Quantization runs in natural [o, i] layout (scale is a per-partition
scalar there; comparisons stay exact fp32 -- only the final big matmul is
f32r/TF32, which rounds operands to ~11 mantissa bits). Phase B is g-major
so the folded W_eff^T [128 (i%128), 32 (i//128), 512 (o)] fills
group-by-group and phase C's accumulation chains can start early. x tiles
are PE-transposed (f32r, 1.5 cyc/row) through PSUM and cast-copied into the
same folded layout; f32r matmuls (1 cyc/row) accumulate out[m(128), o(512)].
"""

import numpy as np
import sys

for _p in ("/opt/trn_rl_repo",):
    if _p not in sys.path:
        sys.path.insert(0, _p)

import concourse.mybir as mybir  # noqa: E402
import concourse.tile as tile  # noqa: E402
from concourse import bacc  # noqa: E402
from concourse.bass_utils import run_bass_kernel_spmd  # noqa: E402
from concourse.masks import make_identity  # noqa: E402
from contextlib import ExitStack  # noqa: E402

N_CORES = 8
M = 8192  # 4 * 2048 tokens
I = 4096  # in_features
O = 4096  # out_features
GROUP = 128
NG = I // GROUP  # 32 groups along i
RANK = 64
ALPHA_OVER_RANK = 32.0 / 64.0
OS = O // N_CORES  # 512 out features per core
NOB = OS // 128  # 4 output row blocks per core
NMB = M // 128  # 64 m blocks

F32 = mybir.dt.float32
F32R = mybir.dt.float32r

_cache = {}


def _build_program(cb0, tk, dk, reps=1):
    """cb0: smallest normalized codebook entry; tk: 3 bucket thresholds;
    dk: 3 successive codebook differences. All host floats baked in."""
    nc = bacc.Bacc("TRN2", target_bir_lowering=False, debug=False)

    x_d = nc.dram_tensor("x", [M, I], F32R, kind="ExternalInput").ap()
    w_d = nc.dram_tensor("w", [OS, I], F32, kind="ExternalInput").ap()
    scl_d = nc.dram_tensor("scl", [OS, NG], F32, kind="ExternalInput").ap()
    la_d = nc.dram_tensor("la", [RANK, I], F32, kind="ExternalInput").ap()
    lbt_d = nc.dram_tensor("lbt", [RANK, OS], F32, kind="ExternalInput").ap()
    out_d = nc.dram_tensor("out", [M, OS], F32, kind="ExternalOutput").ap()

    with tile.TileContext(nc) as tc, ExitStack() as ctx:
        singles = ctx.enter_context(tc.tile_pool(name="singles", bufs=1))

        ident = singles.tile([128, 128], F32)
        make_identity(nc, ident)
        identr = singles.tile([128, 128], F32R)
        nc.vector.tensor_copy(identr, ident)

        # per-partition scale scalars S = exp(scale_log), natural
        # [o%128, ob, g] layout ("scl" already holds exp values -- a
        # sub-ulp host exp keeps quantization decisions aligned with the
        # reference; the on-chip ACT Exp table is ~2e-6 off, which flips
        # buckets near thresholds)
        ssc = singles.tile([128, NOB, NG], F32)  # S
        for ob in range(NOB):
            nc.sync.dma_start(
                out=ssc[:, ob, :], in_=scl_d[ob * 128 : (ob + 1) * 128, :]
            )
        c0sc = singles.tile([128, NOB, NG], F32)  # cb0 * S
        nc.vector.tensor_scalar_mul(c0sc, ssc, float(cb0))
        tsc = []  # t_k * S
        for k in range(3):
            t = singles.tile([128, NOB, NG], F32, tag=f"tsc{k}")
            nc.vector.tensor_scalar_mul(t, ssc, float(tk[k]))
            tsc.append(t)

        la_sb = singles.tile([RANK, I], F32)
        nc.sync.dma_start(out=la_sb, in_=la_d)
        lbt_sb = singles.tile([RANK, OS], F32)
        nc.sync.dma_start(out=lbt_sb, in_=lbt_d)
        # fold alpha/rank into B^T once
        nc.vector.tensor_scalar_mul(lbt_sb, lbt_sb, float(ALPHA_OVER_RANK))

        # persistent effective transposed weight, folded [i%128, g, o], f32r
        weff = singles.tile([128, NG, OS], F32R)

        if reps > 1:
            ctx.enter_context(tc.For_i(0, reps, 1))

        # ---- phase B (g-major): lora + quantize -> transpose -> weff[g] ----
        wload = ctx.enter_context(tc.tile_pool(name="wload", bufs=8))
        qtmp = ctx.enter_context(tc.tile_pool(name="qtmp", bufs=3))
        wq = ctx.enter_context(tc.tile_pool(name="wq", bufs=8))
        psumD = ctx.enter_context(tc.tile_pool(name="psumD", bufs=2, space="PSUM"))
        psumW = ctx.enter_context(tc.tile_pool(name="psumW", bufs=2, space="PSUM"))

        for g in range(NG):
            gsl = slice(g * 128, (g + 1) * 128)
            # lora delta for all 4 o-blocks of this group, fp32-exact
            d_all = psumD.tile([128, NOB, 128], F32, tag="d")
            for ob in range(NOB):
                nc.tensor.matmul(
                    d_all[:, ob, :],
                    lhsT=lbt_sb[:, ob * 128 : (ob + 1) * 128],
                    rhs=la_sb[:, gsl],
                    start=True,
                    stop=True,
                )
            pt = psumW.tile([128, NOB, 128], F32R, tag="pt")
            for ob in range(NOB):
                wn = wload.tile([128, 128], F32, tag="wn")
                nc.sync.dma_start(out=wn, in_=w_d[ob * 128 : (ob + 1) * 128, gsl])
                u = qtmp.tile([128, 128], F32, tag="u")
                nc.vector.tensor_add(u, wn, d_all[:, ob, :])
                a1 = qtmp.tile([128, 128], F32, tag="a1")
                nc.vector.tensor_scalar(
                    a1, u, tsc[0][:, ob, g : g + 1], float(dk[0]),
                    op0=mybir.AluOpType.is_gt, op1=mybir.AluOpType.mult,
                )
                a2 = qtmp.tile([128, 128], F32, tag="a2")
                nc.vector.tensor_scalar(
                    a2, u, tsc[1][:, ob, g : g + 1], float(dk[1]),
                    op0=mybir.AluOpType.is_gt, op1=mybir.AluOpType.mult,
                )
                a3 = qtmp.tile([128, 128], F32, tag="a3")
                nc.vector.tensor_scalar(
                    a3, u, tsc[2][:, ob, g : g + 1], float(dk[2]),
                    op0=mybir.AluOpType.is_gt, op1=mybir.AluOpType.mult,
                )
                # staircase sum on the (otherwise idle) gpsimd engine
                nc.gpsimd.tensor_add(a1, a1, a2)
                nc.gpsimd.tensor_add(a1, a1, a3)
                # m = (q - cb0)*S + cb0*S
                nc.vector.tensor_scalar(
                    a1, a1, ssc[:, ob, g : g + 1], c0sc[:, ob, g : g + 1],
                    op0=mybir.AluOpType.mult, op1=mybir.AluOpType.add,
                )
                # w_eff = q*S + D, rounded to f32r on write
                wqn = wq.tile([128, 128], F32R, tag="wq")
                nc.vector.tensor_add(wqn, a1, d_all[:, ob, :])
                nc.tensor.transpose(pt[:, ob, :], wqn, identr)
            # one cast-copy lands the whole group row of W_eff^T
            nc.scalar.copy(weff[:, g, :], pt.bitcast(F32))

        # ---- phase C: stream x, transpose, matmul --------------------------
        xpool = ctx.enter_context(tc.tile_pool(name="xpool", bufs=2))
        xtpool = ctx.enter_context(tc.tile_pool(name="xtpool", bufs=3))
        opool = ctx.enter_context(tc.tile_pool(name="opool", bufs=3))
        psumT = ctx.enter_context(tc.tile_pool(name="psumT", bufs=2, space="PSUM"))
        psumO = ctx.enter_context(tc.tile_pool(name="psumO", bufs=2, space="PSUM"))

        for mb in range(NMB):
            x_t = xpool.tile([128, I], F32R, tag="x")
            nc.sync.dma_start(out=x_t, in_=x_d[mb * 128 : (mb + 1) * 128, :])

            xT = xtpool.tile([128, NG, 128], F32R, tag="xT")
            for q in range(NG // 4):
                pxt = psumT.tile([128, 4, 128], F32R, tag="pxt")
                for j in range(4):
                    g = q * 4 + j
                    nc.tensor.transpose(
                        pxt[:, j, :], x_t[:, g * 128 : (g + 1) * 128], identr
                    )
                # cast-copy (bitcast input so the verifier sees an f32->f32r
                # rounding op; transpose output doesn't count as rounded)
                if q % 2 == 0:
                    nc.scalar.copy(xT[:, q * 4 : (q + 1) * 4, :], pxt.bitcast(F32))
                else:
                    nc.vector.tensor_copy(xT[:, q * 4 : (q + 1) * 4, :],
                                          pxt.bitcast(F32))

            p_out = psumO.tile([128, OS], F32, tag="p_out")
            for g in range(NG):
                nc.tensor.matmul(
                    p_out,
                    lhsT=xT[:, g, :],
                    rhs=weff[:, g, :],
                    start=(g == 0),
                    stop=(g == NG - 1),
                )

            o_sb = opool.tile([128, OS], F32, tag="o")
            nc.scalar.copy(o_sb, p_out)
            nc.sync.dma_start(out=out_d[mb * 128 : (mb + 1) * 128, :], in_=o_sb)

    nc.compile()
    return nc


def _get_program(cb0, tk, dk, reps=1):
    key = (round(float(cb0), 9), tuple(round(float(t), 9) for t in tk),
           tuple(round(float(d), 9) for d in dk), reps)
    if key not in _cache:
        _cache[key] = _build_program(cb0, tk, dk, reps)
    return _cache[key]


def kernel(x, weight, scale_log, codebook, lora_A, lora_B):
    xf = np.ascontiguousarray(x.reshape(M, I), dtype=np.float32)

    cb = np.asarray(codebook, dtype=np.float64)
    cb = cb / max(float(np.max(np.abs(cb))), 1e-8)
    tk = (cb[:-1] + cb[1:]) * 0.5
    dk = np.diff(cb)

    nc = _get_program(float(cb[0]), [float(v) for v in tk], [float(v) for v in dk])

    in_maps = []
    for c in range(N_CORES):
        sl = slice(c * OS, (c + 1) * OS)
        in_maps.append({
            "x": xf,
            "w": np.ascontiguousarray(weight[sl], dtype=np.float32),
            "scl": np.exp(np.ascontiguousarray(
                scale_log.reshape(O, NG)[sl], dtype=np.float32)),
            "la": np.ascontiguousarray(lora_A, dtype=np.float32),
            "lbt": np.ascontiguousarray(lora_B[sl].T, dtype=np.float32),
        })

    res = run_bass_kernel_spmd(nc, in_maps, core_ids=list(range(N_CORES))).results
    out = np.concatenate([res[c]["out"] for c in range(N_CORES)], axis=1)
    return out.reshape(x.shape[0], x.shape[1], O)



# revision 15
# speedup vs baseline: 1.7380x; 1.7380x over previous
"""CodebookLoRASTELinear forward on 8 Trainium2 NeuronCores.

out = x @ (W_q + D)^T
  D   = (lora_B @ lora_A) * (alpha/rank)
  cb  = codebook / max|codebook|
  S   = exp(scale_log)                     (per [o, i//128] group)
  q   = cb[searchsorted(midpoints(cb), (W+D)/S)]
      == cm + sum_k (d_k/2) * sign((W+D)/S - t_k),  cm = cb0 + sum_k d_k/2
  W_eff = q*S + D

Column-parallel sharding: W / scale / lora_B rows (out_features) are split
across the 8 cores; x and lora_A are replicated; per-core outputs are
concatenated on the host (no collectives).

The host pre-transposes x to x^T [I, M] in bf16, so phase C is pure bf16
matmul (1 cyc/row, no PE transposes of x) and x HBM traffic is halved.

Phase B computes, per pair of i-groups:
  PE:  Dc = D + cm*S (augmented-rank lora matmul, f32r 256-wide moving)
       u' = w + Dc   (same matmul + identity-matmul accumulate of w)
  DVE: v' = u' * invS_rep   (invS replicated along i, streamed from host,
       so thresholds are CONSTANT and every op is a full-tile op)
  ACT: s_k = Sign(v' - (cm + t_k)), k=1..3, output bf16 (+-1 exact)
  DVE: staircase in bf16 (tensor_scalar 4x / tensor_tensor 2x DVE modes):
       w_k = s_k * d_k/2;  a = w1+w2+w3 (= q - cm, exact in bf16);
       m = a * S_rep(bf16);  w_eff = m + Dc  -> bf16
  PE:  transpose w_eff tiles into folded W_eff^T [i%128, g, o]
(one tensor_tensor add is placed on GPSIMD per pair to offload DVE).

Phase C streams x^T chunks and accumulates out[m(128), o(512)] over the 32
i-groups with bf16 matmuls (moving operand 512 wide, 1 cyc/row).
"""

import numpy as np
import sys

for _p in ("/opt/trn_rl_repo",):
    if _p not in sys.path:
        sys.path.insert(0, _p)

import ml_dtypes  # noqa: E402
import concourse.mybir as mybir  # noqa: E402
import concourse.tile as tile  # noqa: E402
from concourse import bacc  # noqa: E402
from concourse.bass_utils import run_bass_kernel_spmd  # noqa: E402
from concourse.masks import make_identity  # noqa: E402
from contextlib import ExitStack  # noqa: E402

N_CORES = 8
M = 8192  # 4 * 2048 tokens
I = 4096  # in_features
O = 4096  # out_features
GROUP = 128
NG = I // GROUP  # 32 groups along i
RANK = 64
KAUG = RANK + NG  # 96: lora rank + one row per group for cm*S
ALPHA_OVER_RANK = 32.0 / 64.0
OS = O // N_CORES  # 512 out features per core
NOB = OS // 128  # 4 output row blocks per core
MC = 256  # phase C m-chunk
NMC = M // MC  # 32 m chunks

F32 = mybir.dt.float32
F32R = mybir.dt.float32r
BF16 = mybir.dt.bfloat16
BF16_NP = ml_dtypes.bfloat16

_cache = {}


def _build_program(cb0, tk, dk, reps=1):
    """cb0: smallest normalized codebook entry; tk: 3 bucket thresholds;
    dk: 3 successive codebook differences. All host floats baked in."""
    nc = bacc.Bacc("TRN2", target_bir_lowering=False, debug=False)

    xt_d = nc.dram_tensor("xt", [I, M], BF16, kind="ExternalInput").ap()
    w_d = nc.dram_tensor("w", [OS, I], F32R, kind="ExternalInput").ap()
    isr_d = nc.dram_tensor("isr", [OS, I], F32, kind="ExternalInput").ap()
    sr_d = nc.dram_tensor("sr", [OS, I], BF16, kind="ExternalInput").ap()
    la_d = nc.dram_tensor("la", [KAUG, I], F32R, kind="ExternalInput").ap()
    lbt_d = nc.dram_tensor("lbt", [KAUG, OS], F32R, kind="ExternalInput").ap()
    out_d = nc.dram_tensor("out", [M, OS], F32, kind="ExternalOutput").ap()

    cm = float(cb0) + float(sum(dk)) * 0.5

    with tile.TileContext(nc) as tc, ExitStack() as ctx:
        singles = ctx.enter_context(tc.tile_pool(name="singles", bufs=1))

        ident = singles.tile([128, 128], F32)
        make_identity(nc, ident)
        identb = singles.tile([128, 128], BF16)
        nc.vector.tensor_copy(identb, ident)
        identr = singles.tile([128, 128], F32R)
        nc.vector.tensor_copy(identr, ident)

        # constant sign biases -(cm + t_k), one per threshold
        bias = []
        for k in range(3):
            b = singles.tile([128, 1], F32, tag=f"bias{k}")
            nc.vector.memset(b, -(cm + float(tk[k])))
            bias.append(b)

        lbt_sb = singles.tile([KAUG, OS], F32R)
        nc.sync.dma_start(out=lbt_sb, in_=lbt_d)
        lbt_r = lbt_sb
        # la in 4 independently-loaded tiles so the first lora matmul only
        # waits on the first quarter
        la_parts, la_rparts = [], []
        for j in range(4):
            lp = singles.tile([KAUG, I // 4], F32R, tag=f"la{j}")
            nc.sync.dma_start(out=lp, in_=la_d[:, j * (I // 4) : (j + 1) * (I // 4)])
            la_parts.append(lp)
            la_rparts.append(lp)

        # persistent effective transposed weight, folded [i%128, g, o], bf16
        weff = singles.tile([128, NG, OS], BF16)

        if reps > 1:
            ctx.enter_context(tc.For_i(0, reps, 1))

        # ---- phase B (g-pair-major) ----------------------------------------
        wload = ctx.enter_context(tc.tile_pool(name="wload", bufs=2))
        ispool = ctx.enter_context(tc.tile_pool(name="ispool", bufs=2))
        srpool = ctx.enter_context(tc.tile_pool(name="srpool", bufs=2))
        vpool = ctx.enter_context(tc.tile_pool(name="vpool", bufs=2))
        spool = ctx.enter_context(tc.tile_pool(name="spool", bufs=2))
        cpool = ctx.enter_context(tc.tile_pool(name="cpool", bufs=2))
        wqpool = ctx.enter_context(tc.tile_pool(name="wq", bufs=2))
        psumA = ctx.enter_context(tc.tile_pool(name="psumA", bufs=2, space="PSUM"))
        psumB = ctx.enter_context(tc.tile_pool(name="psumB", bufs=1, space="PSUM"))
        # transpose staging shares PSUM banks with phase C's output pool
        psumX = ctx.enter_context(tc.tile_pool(name="psumX", bufs=2, space="PSUM"))

        rearr = "(ob p) (gg i) -> p ob gg i"
        NGP = NG // 2

        def b_front(gp):
            """DMA loads + PE matmuls for gpair gp; returns live tiles."""
            g0 = 2 * gp
            gsl = slice(g0 * 128, (g0 + 2) * 128)
            w_sb = wload.tile([128, NOB, 2, 128], F32R, tag="wn")
            nc.sync.dma_start(out=w_sb, in_=w_d[:, gsl].rearrange(rearr, p=128, gg=2))
            is_sb = ispool.tile([128, NOB, 2, 128], F32, tag="is")
            nc.sync.dma_start(out=is_sb, in_=isr_d[:, gsl].rearrange(rearr, p=128, gg=2))
            sr_sb = srpool.tile([128, NOB, 2, 128], BF16, tag="sr")
            nc.sync.dma_start(out=sr_sb, in_=sr_d[:, gsl].rearrange(rearr, p=128, gg=2))

            la_part = la_rparts[gp // 4]
            lsl = slice(g0 * 128 - (gp // 4) * (I // 4),
                        (g0 + 2) * 128 - (gp // 4) * (I // 4))
            # Dc = D + cm*S via augmented-rank f32r matmul (256-wide moving)
            d_a = psumA.tile([128, NOB, 2, 128], F32, tag="da")
            for ob in range(NOB):
                nc.tensor.matmul(
                    d_a[:, ob, :, :],
                    lhsT=lbt_r[:, ob * 128 : (ob + 1) * 128],
                    rhs=la_part[:, lsl],
                    start=True,
                    stop=True,
                )
            # u' = w + Dc: same matmul plus identity-matmul accumulate of w
            d_b = psumB.tile([128, NOB, 2, 128], F32, tag="db")
            for ob in range(NOB):
                nc.tensor.matmul(
                    d_b[:, ob, :, :],
                    lhsT=lbt_r[:, ob * 128 : (ob + 1) * 128],
                    rhs=la_part[:, lsl],
                    start=True,
                    stop=False,
                )
                nc.tensor.matmul(
                    d_b[:, ob, :, :],
                    lhsT=identr,
                    rhs=w_sb[:, ob, :, :],
                    start=False,
                    stop=True,
                )
            return w_sb, is_sb, sr_sb, d_a, d_b

        def b_chain(gp, tiles):
            """Elementwise quantization chain for gpair gp (DVE/ACT/Pool)."""
            w_sb, is_sb, sr_sb, d_a, d_b = tiles
            # v' = u' * invS  (thresholds constant in this domain)
            v = vpool.tile([128, NOB, 2, 128], F32, tag="v")
            nc.vector.tensor_mul(v, d_b, is_sb)

            # s_k = Sign(v' - (cm + t_k)) in bf16 (+-1 exact)
            s1 = spool.tile([128, NOB, 2, 128], BF16, tag="s1")
            nc.scalar.sign(s1, v, bias=bias[0])
            s2 = spool.tile([128, NOB, 2, 128], BF16, tag="s2")
            nc.scalar.sign(s2, v, bias=bias[1])
            s3 = spool.tile([128, NOB, 2, 128], BF16, tag="s3")
            nc.scalar.sign(s3, v, bias=bias[2])

            # bf16 staircase: a = sum_k (d_k/2) s_k  == q - cm (exact)
            w1 = cpool.tile([128, NOB, 2, 128], BF16, tag="w1")
            nc.vector.tensor_scalar_mul(w1, s1, float(dk[0]) * 0.5)
            w2 = cpool.tile([128, NOB, 2, 128], BF16, tag="w2")
            nc.vector.tensor_scalar_mul(w2, s2, float(dk[1]) * 0.5)
            w3 = cpool.tile([128, NOB, 2, 128], BF16, tag="w3")
            nc.vector.tensor_scalar_mul(w3, s3, float(dk[2]) * 0.5)
            a12 = cpool.tile([128, NOB, 2, 128], BF16, tag="a12")
            nc.gpsimd.tensor_add(a12, w1, w2)
            a = cpool.tile([128, NOB, 2, 128], BF16, tag="a")
            nc.vector.tensor_add(a, a12, w3)
            # m = (q - cm) * S
            m = cpool.tile([128, NOB, 2, 128], BF16, tag="m")
            nc.vector.tensor_mul(m, a, sr_sb)
            # w_eff = m + Dc  (Dc = D + cm*S; GPSIMD cannot read PSUM)
            wq = wqpool.tile([128, NOB, 2, 128], BF16, tag="wq")
            nc.vector.tensor_add(wq, m, d_a)
            return wq

        def b_back(gp, wq):
            """Transpose + copy W_eff^T rows for gpair gp."""
            g0 = 2 * gp
            pt = psumX.tile([128, 2, NOB, 128], BF16, tag="ps")
            for gg in range(2):
                for ob in range(NOB):
                    nc.tensor.transpose(pt[:, gg, ob, :], wq[:, ob, gg, :], identb)
            # one copy lands both group rows of W_eff^T; alternate engines
            if gp % 2 == 0:
                nc.scalar.copy(weff[:, g0 : g0 + 2, :], pt)
            else:
                nc.vector.tensor_copy(weff[:, g0 : g0 + 2, :], pt)

        # ---- phase C pools + x prefetch (issued before phase B so the DMA
        # engine fills the first chunks while quantization runs) -------------
        xpool = ctx.enter_context(tc.tile_pool(name="xpool", bufs=2))
        opool = ctx.enter_context(tc.tile_pool(name="opool", bufs=2))
        xt_r = xt_d.rearrange("(g p) m -> p g m", p=128)

        def x_load(t):
            msl = slice(t * MC, (t + 1) * MC)
            xsb = xpool.tile([128, NG, MC], BF16, tag="x")
            nc.sync.dma_start(out=xsb, in_=xt_r[:, :, msl])
            return xsb

        xq = []

        # software pipeline: issue gpair gp+1's PE matmuls before gpair gp's
        # transposes so the in-order PE queue never stalls on the DVE chain
        tiles = b_front(0)
        wq_prev = None
        for gp in range(NGP):
            if wq_prev is not None:
                b_back(gp - 1, wq_prev)
            wq_prev = b_chain(gp, tiles)
            if gp + 1 < NGP:
                tiles = b_front(gp + 1)
            if gp == NGP - 2:
                # prefetch the first x chunks once phase B DMA traffic wanes
                xq = [x_load(0), x_load(1)]
        b_back(NGP - 1, wq_prev)

        # ---- phase C: stream x^T chunks, accumulate out tiles ---------------
        for t in range(NMC):
            msl = slice(t * MC, (t + 1) * MC)
            xsb = xq.pop(0)
            o_sb = opool.tile([128, MC // 128, OS], F32, tag="o")
            for j in range(MC // 128):
                p_out = psumX.tile([128, OS], F32, tag="ps")
                for g in range(NG):
                    nc.tensor.matmul(
                        p_out,
                        lhsT=xsb[:, g, j * 128 : (j + 1) * 128],
                        rhs=weff[:, g, :],
                        start=(g == 0),
                        stop=(g == NG - 1),
                    )
                nc.scalar.copy(o_sb[:, j, :], p_out)
            nc.sync.dma_start(
                out=out_d[msl, :].rearrange("(j p) o -> p j o", p=128),
                in_=o_sb,
            )
            if t + 2 < NMC:
                xq.append(x_load(t + 2))

    nc.compile()
    return nc


def _get_program(cb0, tk, dk, reps=1):
    key = (round(float(cb0), 9), tuple(round(float(t), 9) for t in tk),
           tuple(round(float(d), 9) for d in dk), reps)
    if key not in _cache:
        _cache[key] = _build_program(cb0, tk, dk, reps)
    return _cache[key]


def _make_in_maps(x, weight, scale_log, codebook, lora_A, lora_B):
    cb = np.asarray(codebook, dtype=np.float64)
    cb = cb / max(float(np.max(np.abs(cb))), 1e-8)
    dk = np.diff(cb)
    cm = float(cb[0]) + float(np.sum(dk)) * 0.5

    xb = np.ascontiguousarray(x.reshape(M, I), dtype=np.float32).astype(BF16_NP)
    xt = np.ascontiguousarray(xb.T)

    sl64 = np.exp(np.asarray(scale_log, dtype=np.float64).reshape(O, NG))
    s_full = sl64.astype(np.float32)            # S, f32 (matches on-host exp)
    is_full = (1.0 / sl64).astype(np.float32)   # 1/S, f32
    s_rep = np.repeat(s_full.astype(BF16_NP), GROUP, axis=1)   # [O, I] bf16
    is_rep = np.repeat(is_full, GROUP, axis=1)                 # [O, I] f32

    # indicator rows for the cm*S augmentation
    ind = np.zeros((NG, I), dtype=np.float32)
    for g in range(NG):
        ind[g, g * GROUP : (g + 1) * GROUP] = 1.0
    la_aug = np.concatenate(
        [np.ascontiguousarray(lora_A, dtype=np.float32), ind], axis=0)

    in_maps = []
    for c in range(N_CORES):
        sl = slice(c * OS, (c + 1) * OS)
        lbt = lora_B[sl].T.astype(np.float32) * ALPHA_OVER_RANK  # [64, OS]
        lbt_aug = np.concatenate([lbt, cm * s_full[sl].T], axis=0)  # [96, OS]
        in_maps.append({
            "xt": xt,
            "w": np.ascontiguousarray(weight[sl], dtype=np.float32),
            "isr": np.ascontiguousarray(is_rep[sl]),
            "sr": np.ascontiguousarray(s_rep[sl]),
            "la": la_aug,
            "lbt": np.ascontiguousarray(lbt_aug),
        })
    return in_maps


def kernel(x, weight, scale_log, codebook, lora_A, lora_B):
    cb = np.asarray(codebook, dtype=np.float64)
    cb = cb / max(float(np.max(np.abs(cb))), 1e-8)
    tk = (cb[:-1] + cb[1:]) * 0.5
    dk = np.diff(cb)

    nc = _get_program(float(cb[0]), [float(v) for v in tk], [float(v) for v in dk])
    in_maps = _make_in_maps(x, weight, scale_log, codebook, lora_A, lora_B)

    res = run_bass_kernel_spmd(nc, in_maps, core_ids=list(range(N_CORES))).results
    out = np.concatenate([res[c]["out"] for c in range(N_CORES)], axis=1)
    return out.reshape(x.shape[0], x.shape[1], O)
